# revision 87
# baseline (speedup 1.0000x reference)
"""A3TGCN (attention temporal GCN) on 8 Trainium2 NeuronCores.

Strategy
--------
The reference computes, per timestep t, three GCN convolutions of the form
segment_sum(norm * (x_t @ W)[src]) which commute with the dense projection:
  A_hat @ (x_t @ W) == (A_hat @ x_t) @ W.
All three convs at every timestep therefore share ONE sparse aggregation
Y = A_hat @ X with X = concat_t x_t  (50k x 192), after which the whole
GRU recurrence + attention is purely node-local dense compute.

Sharding: nodes are partitioned across the 8 cores on the dst axis
(6250 real + 22 pad = 6272 per core). Inputs (X, weights) are replicated,
so no halo exchange is needed at runtime; each core gathers the source
rows for its in-edges directly from its local replica via dma_gather.

Host-side one-time preprocessing (graph/weight setup, as in any deployed
GNN): GCN normalization w' = dinv[src]*w*dinv[dst] (the reference marks
this "computed once"), GRU weight folding
  Wz1 = conv_z_w @ lin_z_w[:H],  bz = conv_z_b @ lin_z_w[:H] + lin_z_b
(ditto r/h), and edge bucketing/padding into the fixed device layout.
All O(E*F*T) work — aggregation, GRU, attention, output projection —
runs on device.

Device pipeline per core:
  phase B, per 128-dst block: dma_gather source rows (fp8e4m3, 256B rows
    — ~5e-3 end-to-end rel err vs the 2e-2 gate, half the random-read
    bytes of fp16; int16 indices over two 25088-row segments of X,
    <=1024 idxs/call round-robined over 4 SWDGE queues), build the
    scatter matrix S_w[e, dmod] = w'_e on DVE (fp16 is_equal mask, fp8
    S_w), accumulate Y_block = S_w^T @ Xg in PSUM with fp8 DoubleRow
    matmuls (two 128-edge chunks per instruction at 0.5 cyc/row),
    add the host-folded self-loop term dinv^2*x (static DMA, no gather
    descriptors), transpose 48-feature groups so timestep trios sit at
    partition offsets 0/64.
  phase C, per 512-node tile, two tiles interleaved: 12 GRU steps
    (fp16 matmuls, f32 PSUM), each emitted in three waves — front
    (h_{t-1}/y-dependent matmuls + r/z activations), the PREVIOUS
    step's attention (its h is settled, so the PE never queues behind
    an unfinished activation chain), then back (urh matmul + tanh +
    gate blend, with both gate products formed off the recurrent
    critical path). Attention scores accumulate densely into PSUM rows
    0..11 via one-hot-column att2 matmuls; at tile end one batched exp,
    a K=12 ones-matmul denominator, reciprocal_approx_fast, and the
    projection-commuted weighted sum: out = sum_t (ow^T h_t)*E_t/den
    via per-trio PSUM row groups {0,32,64}, a K=12 E fan-out matmul,
    one DVE multiply, and a gmat group-sum matmul. The constant att2
    bias cancels in softmax and is dropped; the last (mostly padding)
    tile runs at width 128.
"""

import ml_dtypes
import numpy as np

N, F, T, H, OUT = 50000, 16, 12, 128, 16
NCORE = 8
PERCORE = N // NCORE            # 6250 real nodes per core
BLK = 128
NBLK = 49                       # 49 blocks of 128 = 6272 padded nodes/core
SEG = 25088                     # X segment rows (int16-indexable)
ROWE = 256                      # fp16 elements per X row (192 data + pad)
FD = F * T                      # 192
P = 128
NT = 512                        # node-tile width in phase C
NNT = 13                        # node tiles per core (13*512 = 6656 >= 6272)
YW = NNT * NT                   # padded out width


def _host_prep(x, edge_index, edge_weight):
    """Graph normalization + device data layout. Returns per-core arrays."""
    # self-loops are NOT gathered: their diag contribution dinv[n]^2 * x[n]
    # is host-folded into the dense per-block xown tensor (static DMA + one
    # fused add on device), saving ~6% of the gather descriptors
    src = edge_index[0].astype(np.int64)
    dst = edge_index[1].astype(np.int64)
    w = edge_weight.astype(np.float32)

    deg = (np.bincount(dst, weights=w, minlength=N) + 1.0).astype(np.float32)
    dinv = (1.0 / np.sqrt(deg)).astype(np.float32)
    wn = (dinv[src] * w * dinv[dst]).astype(np.float32)

    core = dst // PERCORE
    rloc = dst % PERCORE
    blk = rloc // BLK
    dmod = (rloc % BLK).astype(np.float16)
    seg = (src >= SEG).astype(np.int64)
    idx16 = (src - SEG * seg).astype(np.int16)

    key = (core * NBLK + blk) * 2 + seg
    order = np.argsort(key, kind="stable")
    key_s = key[order]
    cnt = np.bincount(key, minlength=NCORE * NBLK * 2).reshape(NCORE, NBLK, 2)
    # chunks per (block, segment): max over cores so the SPMD graph is shared
    maxc = cnt.max(0)                                          # [NBLK, 2]
    KLO = np.maximum(1, -(-maxc[:, 0] // P))
    KHI = np.maximum(1, -(-maxc[:, 1] // P))
    CK = KLO + KHI
    ck0 = np.zeros(NBLK + 1, np.int64)
    ck0[1:] = np.cumsum(CK)
    # 16-granular index counts (the gather pads the rest with idx 0, w 0)
    N16 = np.maximum(16, -(-maxc // 16) * 16)                  # [NBLK, 2]

    # position of each edge within its (core, blk, seg) group
    gstart = np.zeros(NCORE * NBLK * 2 + 1, np.int64)
    gstart[1:] = np.cumsum(cnt.ravel())
    j = np.arange(len(order)) - gstart[key_s]

    core_s = core[order]
    blk_s = blk[order]
    seg_s = seg[order]
    idx_s = idx16[order]
    dmod_s = dmod[order]
    wn_s = wn[order].astype(np.float16)

    totck = int(ck0[-1])
    # global chunk column and lane for the (dmod, w) slot layout
    gchunk = ck0[blk_s] + KLO[blk_s] * seg_s + j // P
    lane = j % P
    # wrapped idx layout: within a gather group, index k sits at
    # [k%16, k//16]; gather for (blk b, seg s) owns cols [8*(ck0+KLO*s)...)
    icol = 8 * (ck0[blk_s] + KLO[blk_s] * seg_s) + j // 16
    irow = j % 16

    dm_arr, wv_arr, ix_arr = [], [], []
    for c in range(NCORE):
        m = core_s == c
        dmc = np.zeros((P, totck), np.float16)
        wvc = np.zeros((P, totck), np.float16)
        dmc[lane[m], gchunk[m]] = dmod_s[m]
        wvc[lane[m], gchunk[m]] = wn_s[m]
        ixc = np.zeros((16, totck * 8), np.int16)
        ixc[irow[m], icol[m]] = idx_s[m]

        dm_arr.append(dmc)
        wv_arr.append(wvc)
        ix_arr.append(np.tile(ixc, (8, 1)))

    # X replica: row n = x[n] laid out t-major (col = t*F + f), fp8, padded
    # (fp8e4m3 X costs ~5e-3 end-to-end rel err vs the 2e-2 gate, and halves
    # the gather's random-read HBM traffic: 256B rows instead of 512B)
    xf = np.moveaxis(x, 2, 1).reshape(N, FD)
    xt = np.zeros((2 * SEG, ROWE), ml_dtypes.float8_e4m3fn)
    xt[:N, :FD] = xf
    # per-core self-loop tensor: local row r = dinv[g]^2 * x[g] (fp16, full
    # precision x — the diag path skips the fp8 quantization)
    xo_arr = []
    for c in range(NCORE):
        g = c * PERCORE + np.arange(PERCORE)
        xoc = np.zeros((NBLK * BLK, ROWE), np.float16)
        xoc[:PERCORE, :FD] = (dinv[g] ** 2)[:, None] * xf[g]
        xo_arr.append(xoc)
    return (dm_arr, wv_arr, ix_arr, xo_arr, xt[:SEG], xt[SEG:],
            KLO, KHI, CK, ck0, N16)


def _fold_weights(inp):
    """Fold conv into lin weights (the concat trick) and build the padded
    timestep-trio lhsT variants used by phase C."""
    f32 = np.float32
    wz1 = inp["conv_z_w"].astype(f32) @ inp["lin_z_w"].astype(f32)[:H]
    wr1 = inp["conv_r_w"].astype(f32) @ inp["lin_r_w"].astype(f32)[:H]
    wh1 = inp["conv_h_w"].astype(f32) @ inp["lin_h_w"].astype(f32)[:H]
    bz = inp["conv_z_b"].astype(f32) @ inp["lin_z_w"].astype(f32)[:H] + inp["lin_z_b"].astype(f32)
    br = inp["conv_r_b"].astype(f32) @ inp["lin_r_w"].astype(f32)[:H] + inp["lin_r_b"].astype(f32)
    bh = inp["conv_h_b"].astype(f32) @ inp["lin_h_w"].astype(f32)[:H] + inp["lin_h_b"].astype(f32)

    # wp[:, (gate*3+v)*128 : +128]: rows [16v:16v+16] and [64+16v:+16] = W
    wp = np.zeros((P, 9 * P), np.float16)
    for g, w1 in enumerate((wz1, wr1, wh1)):
        for v in range(3):
            col = (g * 3 + v) * P
            wp[16 * v:16 * v + 16, col:col + P] = w1.astype(np.float16)
            wp[64 + 16 * v:64 + 16 * v + 16, col:col + P] = w1.astype(np.float16)
    # gate-h weights: z/r slots unused at full scale; h slot halved twice
    # (r*h = 0.5*h + 0.5*(u_r*h) folds the sigmoid affine into the matmul)
    w2 = np.concatenate([inp["lin_z_w"][H:], inp["lin_r_w"][H:],
                         inp["lin_h_w"][H:] * 0.5], axis=1).astype(np.float16)
    # att2 one-hot-column fan-out: step t's score matmul uses columns
    # [12t, 12t+12) with att2 in column t, accumulating into PSUM row t
    a2oh = np.zeros((H, 12 * T), np.float16)
    for t in range(T):
        a2oh[:, 12 * t + t] = inp["att2_w"].astype(np.float16).reshape(H)
    return dict(
        wp=wp, w2=w2,
        att1w=inp["att1_w"].astype(np.float16),
        att2w=a2oh,
        outw=inp["out_w"].astype(np.float16),
        bz=(bz / 2).reshape(H, 1).astype(f32),
        br=(br / 2).reshape(H, 1).astype(f32),
        bh=bh.reshape(H, 1).astype(f32),
        ab1=inp["att1_b"].astype(f32).reshape(H, 1),
        outb=inp["out_b"].astype(f32).reshape(OUT, 1),
    )


def _build_graph(totck, KLO, KHI, CK, ck0, N16):
    from concourse import bass, bacc, mybir
    import concourse.tile as tile

    fp16 = mybir.dt.float16
    fp8 = mybir.dt.float8e4
    f32 = mybir.dt.float32
    AF = mybir.ActivationFunctionType
    OP = mybir.AluOpType

    nc = bacc.Bacc("TRN2", target_bir_lowering=False, debug=False,
                   num_devices=NCORE, num_swdge_queues=4,
                   dynamic_dma_scratch_size=16384)

    def din(name, shape, dt=fp16):
        return nc.dram_tensor(name, shape, dt, kind="ExternalInput").ap()

    x0 = din("x0", [SEG, ROWE], fp8)
    x1 = din("x1", [SEG, ROWE], fp8)
    xown = din("xown", [NBLK * BLK, ROWE])
    ix = din("ix", [P, 8 * totck], mybir.dt.int16)
    dm = din("dm", [P, totck])
    wv = din("wv", [P, totck])
    wp = din("wp", [P, 9 * P])
    w2 = din("w2", [P, 3 * P])
    att1w = din("att1w", [P, P])
    att2w = din("att2w", [P, 12 * T])
    outw = din("outw", [P, OUT])
    bz = din("bz", [P, 1], f32)
    br = din("br", [P, 1], f32)
    bh = din("bh", [P, 1], f32)
    ab1 = din("ab1", [P, 1], f32)
    outb = din("outb", [OUT, 1], f32)
    iota = din("iota", [P, P])
    ident = din("ident", [P, P])
    ones65 = din("ones65", [65, P])
    brep = din("brep", [T, 4 * P])
    gmat = din("gmat", [P, OUT])
    ones32 = din("ones32", [1, OUT], f32)
    out_d = nc.dram_tensor("out", [OUT, YW], f32, kind="ExternalOutput").ap()

    ckmax = int(CK.max())

    with tile.TileContext(nc) as tc:
        with tc.tile_pool(name="const", bufs=1) as cp, \
             tc.tile_pool(name="work", bufs=1) as wkp, \
             tc.tile_pool(name="ps", bufs=1, space="PSUM") as ps:

            def load(ap_in, shape, dt=fp16, name="c"):
                t = cp.tile(shape, dt, tag=name, name=name)
                nc.sync.dma_start(out=t[:], in_=ap_in[:])
                return t

            # split the edge-table loads so the first pair's gathers and
            # S_w builds only wait on their own slice, not the full 2MB
            spl = int(ck0[8])

            def load2(ap_in, shape, dt, name, col0):
                t = cp.tile(shape, dt, tag=name, name=name)
                nc.sync.dma_start(out=t[:, 0:col0], in_=ap_in[:, 0:col0])
                nc.sync.dma_start(out=t[:, col0:], in_=ap_in[:, col0:])
                return t

            ix_sb = load2(ix, [P, 8 * totck], mybir.dt.int16, "ix", 8 * spl)
            dm_sb = load2(dm, [P, totck], fp16, "dm", spl)
            wv_sb = load2(wv, [P, totck], fp16, "wv", spl)
            wp_sb = load(wp, [P, 9 * P], fp16, "wp")
            w2_sb = load(w2, [P, 3 * P], fp16, "w2")
            a1_sb = load(att1w, [P, P], fp16, "a1")
            a2_sb = load(att2w, [P, 12 * T], fp16, "a2")
            ow_sb = load(outw, [P, OUT], fp16, "ow")
            bz_sb = load(bz, [P, 1], f32, "bz")
            br_sb = load(br, [P, 1], f32, "br")
            bh_sb = load(bh, [P, 1], f32, "bh")
            ab1_sb = load(ab1, [P, 1], f32, "ab1")
            ob_sb = load(outb, [OUT, 1], f32, "ob")
            io_sb = load(iota, [P, P], fp16, "io")
            id_sb = load(ident, [P, P], fp16, "id")
            on_sb = load(ones65, [65, P], fp16, "on")
            brp_sb = load(brep, [T, 4 * P], fp16, "br2")
            g_sb = load(gmat, [P, OUT], fp16, "gm")
            o32_sb = load(ones32, [1, OUT], f32, "o32")

            # zeroed-once fp16 staging for the projected-h groups: only rows
            # [32k, 32k+OUT) are ever rewritten, the rest stay 0 so the
            # tail's big TT never touches PSUM garbage
            phs_t = [cp.tile([P, NT], fp16, tag=f"phs{k}", name=f"phs{k}")
                     for k in range(3)]
            for ph_ in phs_t:
                nc.vector.memset(ph_[:], 0)
            phctr = [0]
            zro_sb = cp.tile([T, 1], f32, tag="zro", name="zro")
            nc.vector.memset(zro_sb[:], 0)

            # persistent Y tiles: per node-tile, trios t0-2/t3-5 at rows 0/64
            y0s = [cp.tile([P, NT], fp16, tag=f"y0_{i}", name=f"y0_{i}")
                   for i in range(NNT)]
            y1s = [cp.tile([P, NT], fp16, tag=f"y1_{i}", name=f"y1_{i}")
                   for i in range(NNT)]
            nc.vector.memset(y0s[12][:], 0)
            nc.vector.memset(y1s[12][:], 0)

            # ---------------- phase B: aggregation ----------------
            qrr = [0]

            def emit_block(b):
                ck = int(CK[b])
                klo = int(KLO[b])
                c0 = int(ck0[b])
                n16lo, n16hi = int(N16[b, 0]), int(N16[b, 1])
                xg = wkp.tile([P, ckmax * ROWE], fp8, tag="xg", bufs=5,
                              name=f"xg{b}")
                if b < 5:
                    nc.vector.memset(xg[:], 0)

                # dma_gather tops out at 1024 indices per call (SWDGE ring);
                # round-robin the 4 SWDGE queues so desc-gen/DMA of
                # adjacent calls overlap
                def gather(src, dst_ck, icol0, nidx):
                    if nidx % P:
                        o = dst_ck + nidx // P
                        nc.vector.memset(xg[:, o * ROWE:(o + 1) * ROWE], 0)
                    done = 0
                    while done < nidx:
                        n = min(1024, nidx - done)
                        o = dst_ck + done // P
                        nck = -(-n // P)
                        q = qrr[0]
                        qrr[0] = (q + 1) % 4
                        nc.gpsimd.dma_gather(
                            out_ap=xg[:, o * ROWE:(o + nck) * ROWE]
                                .rearrange("p (c d) -> p c d", c=nck),
                            in_ap=src[:],
                            idxs_ap=ix_sb[:, 8 * (icol0 + done // P):
                                          8 * (icol0 + done // P) + (n + 15) // 16],
                            num_idxs=n, num_idxs_reg=n,
                            elem_size=ROWE, queue_num=q)
                        done += n

                gather(x0, 0, c0, n16lo)
                gather(x1, klo, c0 + klo, n16hi)

                # S_w[p, c, d] = (dm[p,c] == d) * wv[p,c]; the compare and
                # multiply run at 16-bit (2x DVE), only the final S_w is fp8
                # so the matmuls below can use DoubleRow
                msk = wkp.tile([P, ck * P], fp16, tag="msk", bufs=3,
                               name=f"msk{b}", padded_shape=[P, ckmax * P])
                m3 = msk[:].rearrange("p (c d) -> p c d", c=ck)
                sw = wkp.tile([P, ck * P], fp8, tag="sw", bufs=3,
                              name=f"sw{b}", padded_shape=[P, ckmax * P])
                s3 = sw[:].rearrange("p (c d) -> p c d", c=ck)
                iota_bc = bass.AP(io_sb.tensor, 0, [[P, P], [0, ck], [1, P]])
                nc.vector.tensor_tensor(
                    out=m3, in0=iota_bc,
                    in1=dm_sb[:, c0:c0 + ck].to_broadcast([P, ck, P]),
                    op=OP.is_equal)
                nc.vector.tensor_tensor(
                    out=s3, in0=m3,
                    in1=wv_sb[:, c0:c0 + ck].to_broadcast([P, ck, P]),
                    op=OP.mult)

                # self-loop diag term: host-scaled dinv^2*x rows, static DMA
                xo = wkp.tile([P, FD], fp16, tag="xo", bufs=3, name=f"xo{b}")
                nc.sync.dma_start(out=xo[:], in_=xown[b * P:(b + 1) * P, 0:FD])

                # Y_block[d, f] = sum_c S_c^T @ Xg_c   (node-major);
                # fp8 DoubleRow contracts two 128-edge chunks per instruction
                # at 0.5 cycles/row
                yps = ps.tile([P, FD], f32, tag="pB", name="yps")
                c = 0
                while c < ck:
                    if c + 1 < ck:
                        lhs2 = bass.AP(sw.tensor, c * P,
                                       [[ckmax * P, P], [P, 2], [1, P]])
                        rhs2 = bass.AP(xg.tensor, c * ROWE,
                                       [[ckmax * ROWE, P], [ROWE, 2], [1, FD]])
                        nc.tensor.matmul(
                            out=yps[:], lhsT=lhs2, rhs=rhs2,
                            start=(c == 0), stop=(c + 2 >= ck),
                            perf_mode=mybir.MatmulPerfMode.DoubleRow)
                        c += 2
                    else:
                        nc.tensor.matmul(
                            out=yps[:], lhsT=sw[:, c * P:(c + 1) * P],
                            rhs=xg[:, c * ROWE:c * ROWE + FD],
                            start=(c == 0), stop=True)
                        c += 1
                yb = wkp.tile([P, FD], fp16, tag="yb", bufs=2, name=f"yb{b}")
                nc.vector.scalar_tensor_tensor(
                    out=yb[:], in0=xo[:], scalar=1.0, in1=yps[:],
                    op0=OP.mult, op1=OP.add)

                # transpose 48-col groups to [48, 128] at bases 0/64
                nt_i, csl = b // 4, slice((b % 4) * P, (b % 4 + 1) * P)
                for half, yt in ((0, y0s[nt_i]), (1, y1s[nt_i])):
                    tp = ps.tile([P, P], fp16, tag="pB", name="tt")
                    for g in (2 * half, 2 * half + 1):
                        nc.tensor.transpose(out=tp[64 * (g % 2):64 * (g % 2) + 48, :],
                                            in_=yb[:, g * 48:(g + 1) * 48],
                                            identity=id_sb[:])
                    nc.vector.tensor_copy(out=yt[0:48, csl], in_=tp[0:48, :])
                    nc.vector.tensor_copy(out=yt[64:112, csl], in_=tp[64:112, :])

            # ---------------- phase C: GRU + attention ----------------
            # sigmoid(x) = (1 + tanh(x/2))/2 keeps every ACT function of
            # the kernel (tanh/relu/copy/exp) in ONE activation table.
            def tile_state(i):
                # attention scores accumulate densely in PSUM rows 0..11:
                # step t adds att2^T relu1_t into row t via a one-hot-column
                # lhsT (engines can't write single partitions off 32-alignment,
                # so PE accumulation is the only dense-staging path)
                scs = ps.tile([T, NT], f32, tag="scp", bufs=2, name=f"scp_{i}")
                # the last tile holds only 128 real nodes; narrow every
                # phase-C instruction to w columns (pools stay full-width)
                return dict(i=i, hts=[], scs=scs, w=(NT if i < NNT - 1 else P))

            # Each step is emitted in three waves batched across the
            # interleaved tile pair — front (everything that only needs
            # h_{t-1}/y), the PREVIOUS step's attention (uses h_{t-1}, always
            # ready), then back (the urh-dependent hpre matmul + h update).
            # This keeps the PE's static order free of instructions that
            # wait on the step's own activation chain, so one tile's stall
            # never blocks the other tile's independent matmuls.
            def emit_front(st, t):
                    i, hts, w = st["i"], st["hts"], st["w"]
                    v = t % 3
                    base = 64 * ((t % 6) // 3)
                    yt = (y0s[i] if t < 6 else y1s[i])[base:base + 48, 0:w]
                    hprev = hts[-1] if t else None

                    def ypart(gi, pre, stop):
                        nc.tensor.matmul(
                            out=pre[:, 0:w],
                            lhsT=wp_sb[base:base + 48,
                                       (gi * 3 + v) * P:(gi * 3 + v + 1) * P],
                            rhs=yt, start=True, stop=stop)

                    # h~ = tanh(Wh1.y + (Wh2/2).(h*(1+u_r)) + bh)
                    # (r*h = 0.5*(1+u_r)*h with the 0.5 folded into Wh2; the
                    # (1+u_r)*h product is one fused scalar_tensor_tensor)
                    hpre = ps.tile([P, NT], f32, tag="hp", bufs=2, name="hpre")
                    st["hpre"] = hpre
                    if t:
                        rpre = ps.tile([P, NT], f32, tag="big", bufs=2,
                                       name="rpre")
                        ypart(1, rpre, False)
                        nc.tensor.matmul(out=rpre[:, 0:w], lhsT=w2_sb[:, P:2 * P],
                                         rhs=hprev[:, 0:w], start=False, stop=True)
                        ur = wkp.tile([P, NT], fp16, tag="ur", bufs=3,
                                      name="ur")
                        nc.scalar.activation(out=ur[:, 0:w], in_=rpre[:, 0:w],
                                             func=AF.Tanh, bias=br_sb[:],
                                             scale=0.5)
                        urh = wkp.tile([P, NT], fp16, tag="urh", bufs=3,
                                       name="urh")
                        nc.vector.scalar_tensor_tensor(
                            out=urh[:, 0:w], in0=ur[:, 0:w], scalar=1.0,
                            in1=hprev[:, 0:w], op0=OP.add, op1=OP.mult)
                        st["urh"] = urh
                    ypart(2, hpre, t == 0)

                    # off-path: u_z = tanh((zpre + bz)/2);  z = (1 + u_z)/2
                    zpre = ps.tile([P, NT], f32, tag="big", bufs=2, name="zpre")
                    ypart(0, zpre, t == 0)
                    if t:
                        nc.tensor.matmul(out=zpre[:, 0:w], lhsT=w2_sb[:, 0:P],
                                         rhs=hprev[:, 0:w], start=False, stop=True)
                    uz = wkp.tile([P, NT], fp16, tag="uz", bufs=3, name="uz")
                    nc.scalar.activation(out=uz[:, 0:w], in_=zpre[:, 0:w],
                                         func=AF.Tanh, bias=bz_sb[:], scale=0.5)
                    za = wkp.tile([P, NT], fp16, tag="z", bufs=3, name="za")
                    nc.scalar.activation(out=za[:, 0:w], in_=uz[:, 0:w],
                                         func=AF.Copy, scale=-0.5, bias=0.5)
                    st["za"] = za
                    if t:
                        b_ = wkp.tile([P, NT], fp16, tag="zd", bufs=3,
                                      name="b")
                        nc.vector.scalar_tensor_tensor(
                            out=b_[:, 0:w], in0=uz[:, 0:w], scalar=1.0,
                            in1=hprev[:, 0:w], op0=OP.add, op1=OP.mult)
                        st["b_"] = b_

            def emit_att(st, t):
                    # attention score for step t (h_t already settled);
                    # exp deferred to tile end
                    hts, scs, w = st["hts"], st["scs"], st["w"]
                    apre = ps.tile([P, NT], f32, tag="big", bufs=2, name="apre")
                    nc.tensor.matmul(out=apre[:, 0:w], lhsT=a1_sb[:],
                                     rhs=hts[t][:, 0:w], start=True, stop=True)
                    relu1 = wkp.tile([P, NT], fp16, tag="relu1", bufs=3,
                                     name="relu1")
                    nc.scalar.activation(out=relu1[:, 0:w], in_=apre[:, 0:w],
                                         func=AF.Relu, bias=ab1_sb[:])
                    nc.tensor.matmul(out=scs[:, 0:w],
                                     lhsT=a2_sb[:, 12 * t:12 * t + 12],
                                     rhs=relu1[:, 0:w], start=(t == 0),
                                     stop=(t == T - 1), skip_group_check=True)

            def emit_back(st, t):
                    i, hts, w = st["i"], st["hts"], st["w"]
                    hpre, za = st["hpre"], st["za"]
                    if t:
                        nc.tensor.matmul(out=hpre[:, 0:w], lhsT=w2_sb[:, 2 * P:3 * P],
                                         rhs=st["urh"][:, 0:w], start=False,
                                         stop=True)
                    ht_ = wkp.tile([P, NT], fp16, tag="ht", bufs=3, name="ht")
                    nc.scalar.activation(out=ht_[:, 0:w], in_=hpre[:, 0:w],
                                         func=AF.Tanh, bias=bh_sb[:])

                    # h' = (1-z)*h~ + z*h with z = 0.5*u_z + 0.5; za and b
                    # were formed off-path in front, so only the (1-z)*h~
                    # multiply and the final fused add sit behind the tanh
                    h = wkp.tile([P, NT], fp16, tag=f"h{t}", bufs=3,
                                 name=f"h{t}_{i}")
                    if t == 0:
                        nc.vector.tensor_tensor(out=h[:, 0:w], in0=za[:, 0:w],
                                                in1=ht_[:, 0:w], op=OP.mult)
                    else:
                        a_ = wkp.tile([P, NT], fp16, tag="d", bufs=3, name="a")
                        nc.vector.tensor_tensor(out=a_[:, 0:w], in0=za[:, 0:w],
                                                in1=ht_[:, 0:w], op=OP.mult)
                        # h = a + 0.5*b  (b = (1+u_z)*h_prev = 2*z*h_prev)
                        nc.vector.scalar_tensor_tensor(
                            out=h[:, 0:w], in0=st["b_"][:, 0:w], scalar=0.5,
                            in1=a_[:, 0:w], op0=OP.mult, op1=OP.add)
                    hts.append(h)

            def emit_tail(sts):
                # waved across the tile pair: exp/den/rec/softmax-weighted
                # projection; den and osum borrow the hp PSUM buffers (hpre
                # is dead by tail time).
                for st in sts:
                    i, scs, w = st["i"], st["scs"], st["w"]
                    ets = wkp.tile([T, NT], fp16, tag="et", bufs=3,
                                   name=f"et_{i}")
                    nc.scalar.activation(out=ets[:, 0:w], in_=scs[:, 0:w],
                                         func=AF.Exp, bias=zro_sb[:])
                    st["ets"] = ets
                for st in sts:
                    w = st["w"]
                    den = ps.tile([P, NT], f32, tag="hp", bufs=2, name="den")
                    nc.tensor.matmul(out=den[0:1, 0:w], lhsT=on_sb[0:T, 0:1],
                                     rhs=st["ets"][:, 0:w], start=True,
                                     stop=True)
                    st["den"] = den
                for st in sts:
                    w = st["w"]
                    rec = wkp.tile([1, NT], f32, tag="rec", bufs=3, name="rec")
                    nc.vector.reciprocal_approx_fast(out=rec[:, 0:w],
                                                     in_=st["den"][0:1, 0:w])
                    st["rec"] = rec
                for st in sts:
                    w = st["w"]
                    rbc = ps.tile([OUT, NT], f32, tag="sml", bufs=1, name="rbc")
                    nc.tensor.matmul(out=rbc[:, 0:w], lhsT=o32_sb[:],
                                     rhs=st["rec"][:, 0:w], start=True,
                                     stop=True)
                    rbs = wkp.tile([OUT, NT], f32, tag="rbs", bufs=3,
                                   name="rbs")
                    nc.scalar.activation(out=rbs[:, 0:w], in_=rbc[:, 0:w],
                                         func=AF.Copy)
                    st["rbs"] = rbs
                    st["osum"] = ps.tile([P, NT], f32, tag="hp", bufs=2,
                                         name="osum")
                for j in range(4):
                    for st in sts:
                        hts, ets, w = st["hts"], st["ets"], st["w"]
                        php = ps.tile([P, NT], f32, tag="big", bufs=2,
                                      name="php")
                        for k in range(3):
                            nc.tensor.matmul(
                                out=php[32 * k:32 * k + OUT, 0:w],
                                lhsT=ow_sb[:], rhs=hts[3 * j + k][:, 0:w],
                                start=True, stop=True)
                        atr = ps.tile([P, NT], f32, tag="big", bufs=2,
                                      name="atr")
                        nc.tensor.matmul(out=atr[:, 0:w],
                                         lhsT=brp_sb[:, j * P:(j + 1) * P],
                                         rhs=ets[:, 0:w], start=True,
                                         stop=True)
                        phv = phs_t[phctr[0] % 3]
                        phctr[0] += 1
                        for k in range(3):
                            nc.scalar.activation(
                                out=phv[32 * k:32 * k + OUT, 0:w],
                                in_=php[32 * k:32 * k + OUT, 0:w],
                                func=AF.Copy)
                        prt = wkp.tile([P, NT], fp16, tag="prt", bufs=3,
                                       name="prt")
                        nc.vector.tensor_tensor(out=prt[:, 0:w],
                                                in0=phv[:, 0:w],
                                                in1=atr[:, 0:w], op=OP.mult)
                        nc.tensor.matmul(out=st["osum"][0:OUT, 0:w],
                                         lhsT=g_sb[:], rhs=prt[:, 0:w],
                                         start=(j == 0), stop=(j == 3))
                for st in sts:
                    i, w = st["i"], st["w"]
                    osb = wkp.tile([OUT, NT], f32, tag="osb", bufs=3,
                                   name="osb")
                    nc.vector.tensor_tensor(out=osb[:, 0:w],
                                            in0=st["osum"][0:OUT, 0:w],
                                            in1=st["rbs"][:, 0:w], op=OP.mult)
                    nc.vector.tensor_scalar(out=osb[:, 0:w], in0=osb[:, 0:w],
                                            scalar1=ob_sb[:], scalar2=None,
                                            op0=OP.add)
                    nc.sync.dma_start(out=out_d[:, i * NT:i * NT + w],
                                      in_=osb[:, 0:w])

            # interleave: emit each pair of node-tiles right after their
            # source blocks, alternating the two tiles' GRU steps so every
            # engine's static order has two independent dependency chains
            # in flight (hides per-step latency); phase-C compute overlaps
            # the (Q7-serial) phase-B gathers
            for g0 in range(0, NNT, 2):
                tiles = [i for i in (g0, g0 + 1) if i < NNT]
                for i in tiles:
                    for b in range(4 * i, min(4 * i + 4, NBLK)):
                        emit_block(b)
                sts = [tile_state(i) for i in tiles]
                for t in range(T):
                    for st in sts:
                        emit_front(st, t)
                    if t:
                        for st in sts:
                            emit_att(st, t - 1)
                    for st in sts:
                        emit_back(st, t)
                for st in sts:
                    emit_att(st, T - 1)
                emit_tail(sts)

    nc.finalize()
    return nc


def kernel(**inputs):
    from concourse import bass_utils

    x = np.asarray(inputs["x"], np.float32)
    dm_arr, wv_arr, ix_arr, xo_arr, x0, x1, KLO, KHI, CK, ck0, N16 = _host_prep(
        x, np.asarray(inputs["edge_index"]), np.asarray(inputs["edge_weight"]))
    wts = _fold_weights({k: np.asarray(v) for k, v in inputs.items()})
    totck = int(ck0[-1])

    nc = _build_graph(totck, KLO, KHI, CK, ck0, N16)

    iota = np.broadcast_to(np.arange(P, dtype=np.float16), (P, P)).copy()
    ident = np.eye(P, dtype=np.float16)
    ones65 = np.ones((65, P), np.float16)
    # brep fans E_t [T, NT] out to the ph row layout {0,32,64,96}+[0,OUT)
    # for 4-step group j; gmat sums those row groups back to [OUT, NT]
    brep = np.zeros((T, 4 * P), np.float16)
    for t in range(T):
        j, k = t // 3, t % 3
        brep[t, j * P + 32 * k:j * P + 32 * k + OUT] = 1.0
    gmat = np.zeros((P, OUT), np.float16)
    for k in range(3):
        gmat[32 * k + np.arange(OUT), np.arange(OUT)] = 1.0
    ones32 = np.ones((1, OUT), np.float32)
    shared = dict(x0=x0, x1=x1, iota=iota, ident=ident, ones65=ones65,
                  brep=brep, gmat=gmat, ones32=ones32, **wts)
    in_maps = [dict(ix=ix_arr[c], dm=dm_arr[c], wv=wv_arr[c], xown=xo_arr[c],
                    **shared)
               for c in range(NCORE)]

    res = bass_utils.run_bass_kernel_spmd(
        nc, in_maps, core_ids=list(range(NCORE)))
    kernel._last_results = res
    out = np.concatenate(
        [np.asarray(res.results[c]["out"]).T[:PERCORE] for c in range(NCORE)])
    return np.ascontiguousarray(out, dtype=np.float32)



# revision 88
# speedup vs baseline: 1.0031x; 1.0031x over previous
"""A3TGCN (attention temporal GCN) on 8 Trainium2 NeuronCores.

Strategy
--------
The reference computes, per timestep t, three GCN convolutions of the form
segment_sum(norm * (x_t @ W)[src]) which commute with the dense projection:
  A_hat @ (x_t @ W) == (A_hat @ x_t) @ W.
All three convs at every timestep therefore share ONE sparse aggregation
Y = A_hat @ X with X = concat_t x_t  (50k x 192), after which the whole
GRU recurrence + attention is purely node-local dense compute.

Sharding: nodes are partitioned across the 8 cores on the dst axis
(6250 real + 22 pad = 6272 per core). Inputs (X, weights) are replicated,
so no halo exchange is needed at runtime; each core gathers the source
rows for its in-edges directly from its local replica via dma_gather.

Host-side one-time preprocessing (graph/weight setup, as in any deployed
GNN): GCN normalization w' = dinv[src]*w*dinv[dst] (the reference marks
this "computed once"), GRU weight folding
  Wz1 = conv_z_w @ lin_z_w[:H],  bz = conv_z_b @ lin_z_w[:H] + lin_z_b
(ditto r/h), and edge bucketing/padding into the fixed device layout.
All O(E*F*T) work — aggregation, GRU, attention, output projection —
runs on device.

Device pipeline per core:
  phase B, per 128-dst block: dma_gather source rows (fp8e4m3, 256B rows
    — ~5e-3 end-to-end rel err vs the 2e-2 gate, half the random-read
    bytes of fp16; int16 indices over two 25088-row segments of X,
    <=1024 idxs/call round-robined over 4 SWDGE queues), build the
    scatter matrix S_w[e, dmod] = w'_e on DVE (fp16 is_equal mask, fp8
    S_w), accumulate Y_block = S_w^T @ Xg in PSUM with fp8 DoubleRow
    matmuls (two 128-edge chunks per instruction at 0.5 cyc/row),
    add the host-folded self-loop term dinv^2*x (static DMA, no gather
    descriptors), transpose 48-feature groups so timestep trios sit at
    partition offsets 0/64.
  phase C, per 512-node tile, two tiles interleaved: 12 GRU steps
    (fp16 matmuls, f32 PSUM), each emitted in three waves — front
    (h_{t-1}/y-dependent matmuls + r/z activations), the PREVIOUS
    step's attention (its h is settled, so the PE never queues behind
    an unfinished activation chain), then back (urh matmul + tanh +
    gate blend, with both gate products formed off the recurrent
    critical path). Attention scores accumulate densely into PSUM rows
    0..11 via one-hot-column att2 matmuls; at tile end one batched exp,
    a K=12 ones-matmul denominator, reciprocal_approx_fast, and the
    projection-commuted weighted sum: out = sum_t (ow^T h_t)*E_t/den
    via per-trio PSUM row groups {0,32,64}, a K=12 E fan-out matmul,
    one DVE multiply, and a gmat group-sum matmul. The constant att2
    bias cancels in softmax and is dropped; the last (mostly padding)
    tile runs at width 128.
"""

import ml_dtypes
import numpy as np

N, F, T, H, OUT = 50000, 16, 12, 128, 16
NCORE = 8
PERCORE = N // NCORE            # 6250 real nodes per core
BLK = 128
NBLK = 49                       # 49 blocks of 128 = 6272 padded nodes/core
SEG = 25088                     # X segment rows (int16-indexable)
ROWE = 256                      # fp16 elements per X row (192 data + pad)
FD = F * T                      # 192
P = 128
NT = 512                        # node-tile width in phase C
NNT = 13                        # node tiles per core (13*512 = 6656 >= 6272)
YW = NNT * NT                   # padded out width


def _host_prep(x, edge_index, edge_weight):
    """Graph normalization + device data layout. Returns per-core arrays."""
    # self-loops are NOT gathered: their diag contribution dinv[n]^2 * x[n]
    # is host-folded into the dense per-block xown tensor (static DMA + one
    # fused add on device), saving ~6% of the gather descriptors
    src = edge_index[0].astype(np.int64)
    dst = edge_index[1].astype(np.int64)
    w = edge_weight.astype(np.float32)

    deg = (np.bincount(dst, weights=w, minlength=N) + 1.0).astype(np.float32)
    dinv = (1.0 / np.sqrt(deg)).astype(np.float32)
    wn = (dinv[src] * w * dinv[dst]).astype(np.float32)

    core = dst // PERCORE
    rloc = dst % PERCORE
    blk = rloc // BLK
    dmod = (rloc % BLK).astype(np.float16)
    seg = (src >= SEG).astype(np.int64)
    idx16 = (src - SEG * seg).astype(np.int16)

    key = (core * NBLK + blk) * 2 + seg
    order = np.argsort(key, kind="stable")
    key_s = key[order]
    cnt = np.bincount(key, minlength=NCORE * NBLK * 2).reshape(NCORE, NBLK, 2)
    # chunks per (block, segment): max over cores so the SPMD graph is shared
    maxc = cnt.max(0)                                          # [NBLK, 2]
    KLO = np.maximum(1, -(-maxc[:, 0] // P))
    KHI = np.maximum(1, -(-maxc[:, 1] // P))
    CK = KLO + KHI
    ck0 = np.zeros(NBLK + 1, np.int64)
    ck0[1:] = np.cumsum(CK)
    # 16-granular index counts (the gather pads the rest with idx 0, w 0)
    N16 = np.maximum(16, -(-maxc // 16) * 16)                  # [NBLK, 2]

    # position of each edge within its (core, blk, seg) group
    gstart = np.zeros(NCORE * NBLK * 2 + 1, np.int64)
    gstart[1:] = np.cumsum(cnt.ravel())
    j = np.arange(len(order)) - gstart[key_s]

    core_s = core[order]
    blk_s = blk[order]
    seg_s = seg[order]
    idx_s = idx16[order]
    dmod_s = dmod[order]
    wn_s = wn[order].astype(np.float16)

    totck = int(ck0[-1])
    # global chunk column and lane for the (dmod, w) slot layout
    gchunk = ck0[blk_s] + KLO[blk_s] * seg_s + j // P
    lane = j % P
    # wrapped idx layout: within a gather group, index k sits at
    # [k%16, k//16]; gather for (blk b, seg s) owns cols [8*(ck0+KLO*s)...)
    icol = 8 * (ck0[blk_s] + KLO[blk_s] * seg_s) + j // 16
    irow = j % 16

    dm_arr, wv_arr, ix_arr = [], [], []
    for c in range(NCORE):
        m = core_s == c
        dmc = np.zeros((P, totck), np.float16)
        wvc = np.zeros((P, totck), np.float16)
        dmc[lane[m], gchunk[m]] = dmod_s[m]
        wvc[lane[m], gchunk[m]] = wn_s[m]
        ixc = np.zeros((16, totck * 8), np.int16)
        ixc[irow[m], icol[m]] = idx_s[m]

        dm_arr.append(dmc)
        wv_arr.append(wvc)
        ix_arr.append(np.tile(ixc, (8, 1)))

    # X replica: row n = x[n] laid out t-major (col = t*F + f), fp8, padded
    # (fp8e4m3 X costs ~5e-3 end-to-end rel err vs the 2e-2 gate, and halves
    # the gather's random-read HBM traffic: 256B rows instead of 512B)
    xf = np.moveaxis(x, 2, 1).reshape(N, FD)
    xt = np.zeros((2 * SEG, ROWE), ml_dtypes.float8_e4m3fn)
    xt[:N, :FD] = xf
    # per-core self-loop tensor: local row r = dinv[g]^2 * x[g] (fp16, full
    # precision x — the diag path skips the fp8 quantization)
    xo_arr = []
    for c in range(NCORE):
        g = c * PERCORE + np.arange(PERCORE)
        xoc = np.zeros((NBLK * BLK, ROWE), np.float16)
        xoc[:PERCORE, :FD] = (dinv[g] ** 2)[:, None] * xf[g]
        xo_arr.append(xoc)
    return (dm_arr, wv_arr, ix_arr, xo_arr, xt[:SEG], xt[SEG:],
            KLO, KHI, CK, ck0, N16)


def _fold_weights(inp):
    """Fold conv into lin weights (the concat trick) and build the padded
    timestep-trio lhsT variants used by phase C."""
    f32 = np.float32
    wz1 = inp["conv_z_w"].astype(f32) @ inp["lin_z_w"].astype(f32)[:H]
    wr1 = inp["conv_r_w"].astype(f32) @ inp["lin_r_w"].astype(f32)[:H]
    wh1 = inp["conv_h_w"].astype(f32) @ inp["lin_h_w"].astype(f32)[:H]
    bz = inp["conv_z_b"].astype(f32) @ inp["lin_z_w"].astype(f32)[:H] + inp["lin_z_b"].astype(f32)
    br = inp["conv_r_b"].astype(f32) @ inp["lin_r_w"].astype(f32)[:H] + inp["lin_r_b"].astype(f32)
    bh = inp["conv_h_b"].astype(f32) @ inp["lin_h_w"].astype(f32)[:H] + inp["lin_h_b"].astype(f32)

    # wp[:, (gate*3+v)*128 : +128]: rows [16v:16v+16] and [64+16v:+16] = W
    wp = np.zeros((P, 9 * P), np.float16)
    for g, w1 in enumerate((wz1, wr1, wh1)):
        for v in range(3):
            col = (g * 3 + v) * P
            wp[16 * v:16 * v + 16, col:col + P] = w1.astype(np.float16)
            wp[64 + 16 * v:64 + 16 * v + 16, col:col + P] = w1.astype(np.float16)
    # gate-h weights: z/r slots unused at full scale; h slot halved twice
    # (r*h = 0.5*h + 0.5*(u_r*h) folds the sigmoid affine into the matmul)
    w2 = np.concatenate([inp["lin_z_w"][H:], inp["lin_r_w"][H:],
                         inp["lin_h_w"][H:] * 0.5], axis=1).astype(np.float16)
    # att2 one-hot-column fan-out: step t's score matmul uses columns
    # [12t, 12t+12) with att2 in column t, accumulating into PSUM row t
    a2oh = np.zeros((H, 12 * T), np.float16)
    for t in range(T):
        a2oh[:, 12 * t + t] = inp["att2_w"].astype(np.float16).reshape(H)
    return dict(
        wp=wp, w2=w2,
        att1w=inp["att1_w"].astype(np.float16),
        att2w=a2oh,
        outw=inp["out_w"].astype(np.float16),
        bz=(bz / 2).reshape(H, 1).astype(f32),
        br=(br / 2).reshape(H, 1).astype(f32),
        bh=bh.reshape(H, 1).astype(f32),
        ab1=inp["att1_b"].astype(f32).reshape(H, 1),
        outb=inp["out_b"].astype(f32).reshape(OUT, 1),
    )


def _build_graph(totck, KLO, KHI, CK, ck0, N16):
    from concourse import bass, bacc, mybir
    import concourse.tile as tile

    fp16 = mybir.dt.float16
    fp8 = mybir.dt.float8e4
    f32 = mybir.dt.float32
    AF = mybir.ActivationFunctionType
    OP = mybir.AluOpType

    nc = bacc.Bacc("TRN2", target_bir_lowering=False, debug=False,
                   num_devices=NCORE, num_swdge_queues=4,
                   dynamic_dma_scratch_size=16384)

    def din(name, shape, dt=fp16):
        return nc.dram_tensor(name, shape, dt, kind="ExternalInput").ap()

    x0 = din("x0", [SEG, ROWE], fp8)
    x1 = din("x1", [SEG, ROWE], fp8)
    xown = din("xown", [NBLK * BLK, ROWE])
    ix = din("ix", [P, 8 * totck], mybir.dt.int16)
    dm = din("dm", [P, totck])
    wv = din("wv", [P, totck])
    wp = din("wp", [P, 9 * P])
    w2 = din("w2", [P, 3 * P])
    att1w = din("att1w", [P, P])
    att2w = din("att2w", [P, 12 * T])
    outw = din("outw", [P, OUT])
    bz = din("bz", [P, 1], f32)
    br = din("br", [P, 1], f32)
    bh = din("bh", [P, 1], f32)
    ab1 = din("ab1", [P, 1], f32)
    outb = din("outb", [OUT, 1], f32)
    iota = din("iota", [P, P])
    ident = din("ident", [P, P])
    ones65 = din("ones65", [65, P])
    brep = din("brep", [T, 4 * P])
    gmat = din("gmat", [P, OUT])
    ones32 = din("ones32", [1, OUT], f32)
    out_d = nc.dram_tensor("out", [OUT, YW], f32, kind="ExternalOutput").ap()

    ckmax = int(CK.max())

    with tile.TileContext(nc) as tc:
        with tc.tile_pool(name="const", bufs=1) as cp, \
             tc.tile_pool(name="work", bufs=1) as wkp, \
             tc.tile_pool(name="ps", bufs=1, space="PSUM") as ps:

            def load(ap_in, shape, dt=fp16, name="c"):
                t = cp.tile(shape, dt, tag=name, name=name)
                nc.sync.dma_start(out=t[:], in_=ap_in[:])
                return t

            ix_sb = load(ix, [P, 8 * totck], mybir.dt.int16, "ix")
            dm_sb = load(dm, [P, totck], fp16, "dm")
            wv_sb = load(wv, [P, totck], fp16, "wv")
            wp_sb = load(wp, [P, 9 * P], fp16, "wp")
            w2_sb = load(w2, [P, 3 * P], fp16, "w2")
            a1_sb = load(att1w, [P, P], fp16, "a1")
            a2_sb = load(att2w, [P, 12 * T], fp16, "a2")
            ow_sb = load(outw, [P, OUT], fp16, "ow")
            bz_sb = load(bz, [P, 1], f32, "bz")
            br_sb = load(br, [P, 1], f32, "br")
            bh_sb = load(bh, [P, 1], f32, "bh")
            ab1_sb = load(ab1, [P, 1], f32, "ab1")
            ob_sb = load(outb, [OUT, 1], f32, "ob")
            io_sb = load(iota, [P, P], fp16, "io")
            id_sb = load(ident, [P, P], fp16, "id")
            on_sb = load(ones65, [65, P], fp16, "on")
            brp_sb = load(brep, [T, 4 * P], fp16, "br2")
            g_sb = load(gmat, [P, OUT], fp16, "gm")
            o32_sb = load(ones32, [1, OUT], f32, "o32")

            # zeroed-once fp16 staging for the projected-h groups: only rows
            # [32k, 32k+OUT) are ever rewritten, the rest stay 0 so the
            # tail's big TT never touches PSUM garbage
            phs_t = [cp.tile([P, NT], fp16, tag=f"phs{k}", name=f"phs{k}")
                     for k in range(3)]
            for ph_ in phs_t:
                nc.vector.memset(ph_[:], 0)
            phctr = [0]
            zro_sb = cp.tile([T, 1], f32, tag="zro", name="zro")
            nc.vector.memset(zro_sb[:], 0)

            # persistent Y tiles: per node-tile, trios t0-2/t3-5 at rows 0/64
            y0s = [cp.tile([P, NT], fp16, tag=f"y0_{i}", name=f"y0_{i}")
                   for i in range(NNT)]
            y1s = [cp.tile([P, NT], fp16, tag=f"y1_{i}", name=f"y1_{i}")
                   for i in range(NNT)]
            nc.vector.memset(y0s[12][:], 0)
            nc.vector.memset(y1s[12][:], 0)

            # ---------------- phase B: aggregation ----------------
            qrr = [0]

            def emit_block(b):
                ck = int(CK[b])
                klo = int(KLO[b])
                c0 = int(ck0[b])
                n16lo, n16hi = int(N16[b, 0]), int(N16[b, 1])
                xg = wkp.tile([P, ckmax * ROWE], fp8, tag="xg", bufs=5,
                              name=f"xg{b}")
                if b < 5:
                    nc.vector.memset(xg[:], 0)

                # dma_gather tops out at 1024 indices per call (SWDGE ring);
                # round-robin the 4 SWDGE queues so desc-gen/DMA of
                # adjacent calls overlap
                def gather(src, dst_ck, icol0, nidx):
                    if nidx % P:
                        o = dst_ck + nidx // P
                        nc.vector.memset(xg[:, o * ROWE:(o + 1) * ROWE], 0)
                    done = 0
                    while done < nidx:
                        n = min(1024, nidx - done)
                        o = dst_ck + done // P
                        nck = -(-n // P)
                        q = qrr[0]
                        qrr[0] = (q + 1) % 4
                        nc.gpsimd.dma_gather(
                            out_ap=xg[:, o * ROWE:(o + nck) * ROWE]
                                .rearrange("p (c d) -> p c d", c=nck),
                            in_ap=src[:],
                            idxs_ap=ix_sb[:, 8 * (icol0 + done // P):
                                          8 * (icol0 + done // P) + (n + 15) // 16],
                            num_idxs=n, num_idxs_reg=n,
                            elem_size=ROWE, queue_num=q)
                        done += n

                gather(x0, 0, c0, n16lo)
                gather(x1, klo, c0 + klo, n16hi)

                # S_w[p, c, d] = (dm[p,c] == d) * wv[p,c]; the compare and
                # multiply run at 16-bit (2x DVE), only the final S_w is fp8
                # so the matmuls below can use DoubleRow
                msk = wkp.tile([P, ck * P], fp16, tag="msk", bufs=3,
                               name=f"msk{b}", padded_shape=[P, ckmax * P])
                m3 = msk[:].rearrange("p (c d) -> p c d", c=ck)
                sw = wkp.tile([P, ck * P], fp8, tag="sw", bufs=3,
                              name=f"sw{b}", padded_shape=[P, ckmax * P])
                s3 = sw[:].rearrange("p (c d) -> p c d", c=ck)
                iota_bc = bass.AP(io_sb.tensor, 0, [[P, P], [0, ck], [1, P]])
                nc.vector.tensor_tensor(
                    out=m3, in0=iota_bc,
                    in1=dm_sb[:, c0:c0 + ck].to_broadcast([P, ck, P]),
                    op=OP.is_equal)
                nc.vector.tensor_tensor(
                    out=s3, in0=m3,
                    in1=wv_sb[:, c0:c0 + ck].to_broadcast([P, ck, P]),
                    op=OP.mult)

                # self-loop diag term: host-scaled dinv^2*x rows, static DMA
                xo = wkp.tile([P, FD], fp16, tag="xo", bufs=3, name=f"xo{b}")
                nc.sync.dma_start(out=xo[:], in_=xown[b * P:(b + 1) * P, 0:FD])

                # Y_block[d, f] = sum_c S_c^T @ Xg_c   (node-major);
                # fp8 DoubleRow contracts two 128-edge chunks per instruction
                # at 0.5 cycles/row
                yps = ps.tile([P, FD], f32, tag="pB", name="yps")
                c = 0
                while c < ck:
                    if c + 1 < ck:
                        lhs2 = bass.AP(sw.tensor, c * P,
                                       [[ckmax * P, P], [P, 2], [1, P]])
                        rhs2 = bass.AP(xg.tensor, c * ROWE,
                                       [[ckmax * ROWE, P], [ROWE, 2], [1, FD]])
                        nc.tensor.matmul(
                            out=yps[:], lhsT=lhs2, rhs=rhs2,
                            start=(c == 0), stop=(c + 2 >= ck),
                            perf_mode=mybir.MatmulPerfMode.DoubleRow)
                        c += 2
                    else:
                        nc.tensor.matmul(
                            out=yps[:], lhsT=sw[:, c * P:(c + 1) * P],
                            rhs=xg[:, c * ROWE:c * ROWE + FD],
                            start=(c == 0), stop=True)
                        c += 1
                yb = wkp.tile([P, FD], fp16, tag="yb", bufs=2, name=f"yb{b}")
                nc.vector.scalar_tensor_tensor(
                    out=yb[:], in0=xo[:], scalar=1.0, in1=yps[:],
                    op0=OP.mult, op1=OP.add)

                # transpose 48-col groups to [48, 128] at bases 0/64
                nt_i, csl = b // 4, slice((b % 4) * P, (b % 4 + 1) * P)
                for half, yt in ((0, y0s[nt_i]), (1, y1s[nt_i])):
                    tp = ps.tile([P, P], fp16, tag="pB", name="tt")
                    for g in (2 * half, 2 * half + 1):
                        nc.tensor.transpose(out=tp[64 * (g % 2):64 * (g % 2) + 48, :],
                                            in_=yb[:, g * 48:(g + 1) * 48],
                                            identity=id_sb[:])
                    nc.vector.tensor_copy(out=yt[0:48, csl], in_=tp[0:48, :])
                    nc.vector.tensor_copy(out=yt[64:112, csl], in_=tp[64:112, :])

            # ---------------- phase C: GRU + attention ----------------
            # sigmoid(x) = (1 + tanh(x/2))/2 keeps every ACT function of
            # the kernel (tanh/relu/copy/exp) in ONE activation table.
            def tile_state(i):
                # attention scores accumulate densely in PSUM rows 0..11:
                # step t adds att2^T relu1_t into row t via a one-hot-column
                # lhsT (engines can't write single partitions off 32-alignment,
                # so PE accumulation is the only dense-staging path)
                scs = ps.tile([T, NT], f32, tag="scp", bufs=2, name=f"scp_{i}")
                # the last tile holds only 128 real nodes; narrow every
                # phase-C instruction to w columns (pools stay full-width)
                return dict(i=i, hts=[], scs=scs, w=(NT if i < NNT - 1 else P))

            # Each step is emitted in three waves batched across the
            # interleaved tile pair — front (everything that only needs
            # h_{t-1}/y), the PREVIOUS step's attention (uses h_{t-1}, always
            # ready), then back (the urh-dependent hpre matmul + h update).
            # This keeps the PE's static order free of instructions that
            # wait on the step's own activation chain, so one tile's stall
            # never blocks the other tile's independent matmuls.
            def emit_front(st, t):
                    i, hts, w = st["i"], st["hts"], st["w"]
                    v = t % 3
                    base = 64 * ((t % 6) // 3)
                    yt = (y0s[i] if t < 6 else y1s[i])[base:base + 48, 0:w]
                    hprev = hts[-1] if t else None

                    def ypart(gi, pre, stop):
                        nc.tensor.matmul(
                            out=pre[:, 0:w],
                            lhsT=wp_sb[base:base + 48,
                                       (gi * 3 + v) * P:(gi * 3 + v + 1) * P],
                            rhs=yt, start=True, stop=stop)

                    # h~ = tanh(Wh1.y + (Wh2/2).(h*(1+u_r)) + bh)
                    # (r*h = 0.5*(1+u_r)*h with the 0.5 folded into Wh2; the
                    # (1+u_r)*h product is one fused scalar_tensor_tensor)
                    hpre = ps.tile([P, NT], f32, tag="hp", bufs=2, name="hpre")
                    st["hpre"] = hpre
                    if t:
                        rpre = ps.tile([P, NT], f32, tag="big", bufs=2,
                                       name="rpre")
                        ypart(1, rpre, False)
                        nc.tensor.matmul(out=rpre[:, 0:w], lhsT=w2_sb[:, P:2 * P],
                                         rhs=hprev[:, 0:w], start=False, stop=True)
                        ur = wkp.tile([P, NT], fp16, tag="ur", bufs=3,
                                      name="ur")
                        nc.scalar.activation(out=ur[:, 0:w], in_=rpre[:, 0:w],
                                             func=AF.Tanh, bias=br_sb[:],
                                             scale=0.5)
                        urh = wkp.tile([P, NT], fp16, tag="urh", bufs=3,
                                       name="urh")
                        nc.vector.scalar_tensor_tensor(
                            out=urh[:, 0:w], in0=ur[:, 0:w], scalar=1.0,
                            in1=hprev[:, 0:w], op0=OP.add, op1=OP.mult)
                        st["urh"] = urh
                    ypart(2, hpre, t == 0)

                    # off-path: u_z = tanh((zpre + bz)/2);  z = (1 + u_z)/2
                    zpre = ps.tile([P, NT], f32, tag="big", bufs=2, name="zpre")
                    ypart(0, zpre, t == 0)
                    if t:
                        nc.tensor.matmul(out=zpre[:, 0:w], lhsT=w2_sb[:, 0:P],
                                         rhs=hprev[:, 0:w], start=False, stop=True)
                    uz = wkp.tile([P, NT], fp16, tag="uz", bufs=3, name="uz")
                    nc.scalar.activation(out=uz[:, 0:w], in_=zpre[:, 0:w],
                                         func=AF.Tanh, bias=bz_sb[:], scale=0.5)
                    za = wkp.tile([P, NT], fp16, tag="z", bufs=3, name="za")
                    nc.scalar.activation(out=za[:, 0:w], in_=uz[:, 0:w],
                                         func=AF.Copy, scale=-0.5, bias=0.5)
                    st["za"] = za
                    if t:
                        b_ = wkp.tile([P, NT], fp16, tag="zd", bufs=3,
                                      name="b")
                        nc.vector.scalar_tensor_tensor(
                            out=b_[:, 0:w], in0=uz[:, 0:w], scalar=1.0,
                            in1=hprev[:, 0:w], op0=OP.add, op1=OP.mult)
                        st["b_"] = b_

            def emit_att(st, t):
                    # attention score for step t (h_t already settled);
                    # exp deferred to tile end
                    hts, scs, w = st["hts"], st["scs"], st["w"]
                    apre = ps.tile([P, NT], f32, tag="big", bufs=2, name="apre")
                    nc.tensor.matmul(out=apre[:, 0:w], lhsT=a1_sb[:],
                                     rhs=hts[t][:, 0:w], start=True, stop=True)
                    relu1 = wkp.tile([P, NT], fp16, tag="relu1", bufs=3,
                                     name="relu1")
                    nc.scalar.activation(out=relu1[:, 0:w], in_=apre[:, 0:w],
                                         func=AF.Relu, bias=ab1_sb[:])
                    nc.tensor.matmul(out=scs[:, 0:w],
                                     lhsT=a2_sb[:, 12 * t:12 * t + 12],
                                     rhs=relu1[:, 0:w], start=(t == 0),
                                     stop=(t == T - 1), skip_group_check=True)

            def emit_back(st, t):
                    i, hts, w = st["i"], st["hts"], st["w"]
                    hpre, za = st["hpre"], st["za"]
                    if t:
                        nc.tensor.matmul(out=hpre[:, 0:w], lhsT=w2_sb[:, 2 * P:3 * P],
                                         rhs=st["urh"][:, 0:w], start=False,
                                         stop=True)
                    ht_ = wkp.tile([P, NT], fp16, tag="ht", bufs=3, name="ht")
                    nc.scalar.activation(out=ht_[:, 0:w], in_=hpre[:, 0:w],
                                         func=AF.Tanh, bias=bh_sb[:])

                    # h' = (1-z)*h~ + z*h with z = 0.5*u_z + 0.5; za and b
                    # were formed off-path in front, so only the (1-z)*h~
                    # multiply and the final fused add sit behind the tanh
                    h = wkp.tile([P, NT], fp16, tag=f"h{t}", bufs=3,
                                 name=f"h{t}_{i}")
                    if t == 0:
                        nc.vector.tensor_tensor(out=h[:, 0:w], in0=za[:, 0:w],
                                                in1=ht_[:, 0:w], op=OP.mult)
                    else:
                        a_ = wkp.tile([P, NT], fp16, tag="d", bufs=3, name="a")
                        nc.vector.tensor_tensor(out=a_[:, 0:w], in0=za[:, 0:w],
                                                in1=ht_[:, 0:w], op=OP.mult)
                        # h = a + 0.5*b  (b = (1+u_z)*h_prev = 2*z*h_prev)
                        nc.vector.scalar_tensor_tensor(
                            out=h[:, 0:w], in0=st["b_"][:, 0:w], scalar=0.5,
                            in1=a_[:, 0:w], op0=OP.mult, op1=OP.add)
                    hts.append(h)

            def emit_tail(sts):
                # waved across the tile pair: exp/den/rec/softmax-weighted
                # projection; den and osum borrow the hp PSUM buffers (hpre
                # is dead by tail time).
                for st in sts:
                    i, scs, w = st["i"], st["scs"], st["w"]
                    ets = wkp.tile([T, NT], fp16, tag="et", bufs=3,
                                   name=f"et_{i}")
                    nc.scalar.activation(out=ets[:, 0:w], in_=scs[:, 0:w],
                                         func=AF.Exp, bias=zro_sb[:])
                    st["ets"] = ets
                for st in sts:
                    w = st["w"]
                    den = ps.tile([P, NT], f32, tag="hp", bufs=2, name="den")
                    nc.tensor.matmul(out=den[0:1, 0:w], lhsT=on_sb[0:T, 0:1],
                                     rhs=st["ets"][:, 0:w], start=True,
                                     stop=True)
                    st["den"] = den
                for st in sts:
                    w = st["w"]
                    rec = wkp.tile([1, NT], f32, tag="rec", bufs=3, name="rec")
                    nc.vector.reciprocal_approx_fast(out=rec[:, 0:w],
                                                     in_=st["den"][0:1, 0:w])
                    st["rec"] = rec
                for st in sts:
                    w = st["w"]
                    rbc = ps.tile([OUT, NT], f32, tag="sml", bufs=1, name="rbc")
                    nc.tensor.matmul(out=rbc[:, 0:w], lhsT=o32_sb[:],
                                     rhs=st["rec"][:, 0:w], start=True,
                                     stop=True)
                    rbs = wkp.tile([OUT, NT], f32, tag="rbs", bufs=3,
                                   name="rbs")
                    nc.scalar.activation(out=rbs[:, 0:w], in_=rbc[:, 0:w],
                                         func=AF.Copy)
                    st["rbs"] = rbs
                    st["osum"] = ps.tile([P, NT], f32, tag="hp", bufs=2,
                                         name="osum")
                for j in range(4):
                    for st in sts:
                        hts, ets, w = st["hts"], st["ets"], st["w"]
                        php = ps.tile([P, NT], f32, tag="big", bufs=2,
                                      name="php")
                        for k in range(3):
                            nc.tensor.matmul(
                                out=php[32 * k:32 * k + OUT, 0:w],
                                lhsT=ow_sb[:], rhs=hts[3 * j + k][:, 0:w],
                                start=True, stop=True)
                        atr = ps.tile([P, NT], f32, tag="big", bufs=2,
                                      name="atr")
                        nc.tensor.matmul(out=atr[:, 0:w],
                                         lhsT=brp_sb[:, j * P:(j + 1) * P],
                                         rhs=ets[:, 0:w], start=True,
                                         stop=True)
                        phv = phs_t[phctr[0] % 3]
                        phctr[0] += 1
                        for k in range(3):
                            nc.scalar.activation(
                                out=phv[32 * k:32 * k + OUT, 0:w],
                                in_=php[32 * k:32 * k + OUT, 0:w],
                                func=AF.Copy)
                        prt = wkp.tile([P, NT], fp16, tag="prt", bufs=3,
                                       name="prt")
                        nc.vector.tensor_tensor(out=prt[:, 0:w],
                                                in0=phv[:, 0:w],
                                                in1=atr[:, 0:w], op=OP.mult)
                        nc.tensor.matmul(out=st["osum"][0:OUT, 0:w],
                                         lhsT=g_sb[:], rhs=prt[:, 0:w],
                                         start=(j == 0), stop=(j == 3))
                for st in sts:
                    i, w = st["i"], st["w"]
                    osb = wkp.tile([OUT, NT], f32, tag="osb", bufs=3,
                                   name="osb")
                    nc.vector.tensor_tensor(out=osb[:, 0:w],
                                            in0=st["osum"][0:OUT, 0:w],
                                            in1=st["rbs"][:, 0:w], op=OP.mult)
                    nc.vector.tensor_scalar(out=osb[:, 0:w], in0=osb[:, 0:w],
                                            scalar1=ob_sb[:], scalar2=None,
                                            op0=OP.add)
                    nc.sync.dma_start(out=out_d[:, i * NT:i * NT + w],
                                      in_=osb[:, 0:w])

            # interleave: emit each pair of node-tiles right after their
            # source blocks, alternating the two tiles' GRU steps so every
            # engine's static order has two independent dependency chains
            # in flight (hides per-step latency); phase-C compute overlaps
            # the (Q7-serial) phase-B gathers
            for g0 in range(0, NNT, 2):
                tiles = [i for i in (g0, g0 + 1) if i < NNT]
                for i in tiles:
                    for b in range(4 * i, min(4 * i + 4, NBLK)):
                        emit_block(b)
                sts = [tile_state(i) for i in tiles]
                for t in range(T):
                    for st in sts:
                        emit_front(st, t)
                    if t:
                        for st in sts:
                            emit_att(st, t - 1)
                    for st in sts:
                        emit_back(st, t)
                for st in sts:
                    emit_att(st, T - 1)
                emit_tail(sts)

    nc.finalize()
    return nc


def kernel(**inputs):
    from concourse import bass_utils

    x = np.asarray(inputs["x"], np.float32)
    dm_arr, wv_arr, ix_arr, xo_arr, x0, x1, KLO, KHI, CK, ck0, N16 = _host_prep(
        x, np.asarray(inputs["edge_index"]), np.asarray(inputs["edge_weight"]))
    wts = _fold_weights({k: np.asarray(v) for k, v in inputs.items()})
    totck = int(ck0[-1])

    nc = _build_graph(totck, KLO, KHI, CK, ck0, N16)

    iota = np.broadcast_to(np.arange(P, dtype=np.float16), (P, P)).copy()
    ident = np.eye(P, dtype=np.float16)
    ones65 = np.ones((65, P), np.float16)
    # brep fans E_t [T, NT] out to the ph row layout {0,32,64,96}+[0,OUT)
    # for 4-step group j; gmat sums those row groups back to [OUT, NT]
    brep = np.zeros((T, 4 * P), np.float16)
    for t in range(T):
        j, k = t // 3, t % 3
        brep[t, j * P + 32 * k:j * P + 32 * k + OUT] = 1.0
    gmat = np.zeros((P, OUT), np.float16)
    for k in range(3):
        gmat[32 * k + np.arange(OUT), np.arange(OUT)] = 1.0
    ones32 = np.ones((1, OUT), np.float32)
    shared = dict(x0=x0, x1=x1, iota=iota, ident=ident, ones65=ones65,
                  brep=brep, gmat=gmat, ones32=ones32, **wts)
    in_maps = [dict(ix=ix_arr[c], dm=dm_arr[c], wv=wv_arr[c], xown=xo_arr[c],
                    **shared)
               for c in range(NCORE)]

    res = bass_utils.run_bass_kernel_spmd(
        nc, in_maps, core_ids=list(range(NCORE)))
    kernel._last_results = res
    out = np.concatenate(
        [np.asarray(res.results[c]["out"]).T[:PERCORE] for c in range(NCORE)])
    return np.ascontiguousarray(out, dtype=np.float32)

